# revision 55
# baseline (speedup 1.0000x reference)
"""CompressedLinear (int8 weight, per-row scale) on 8 Trainium2 NeuronCores.

Math: y[b,s,o] = sum_i x[b,s,i] * (w_int8[o,i] * scale[o]) + bias[o]

Strategy (tensor-parallel over out_features, per sharding hint):
  - Shard W/scale/bias rows across 8 cores (1376 rows each); x replicated.
  - Scale is applied to the matmul OUTPUT (algebraically identical), so the
    device matmuls run on the raw int8 weights cast to fp16 (int8 is exact
    in fp16).
  - Single fp16 matmul pass: int8 weights are exact in fp16, and casting x
    to fp16 bounds the output relative error at ~2e-4 (Frobenius) while
    halving the PE work vs a hi/lo two-pass scheme. (fp8 DoubleRow cannot
    beat this: representing int8 W exactly needs 2 fp8 passes, and e4m3 x
    alone costs ~3.6% error — over the 2e-2 gate — so a 3rd pass would be
    needed; 2+ passes at the measured 1.44x lose to 1 pass at 1x.)
  - Each core computes yT[o_shard, s] = W_shard @ x^T; both operands need
    the contraction dim on SBUF partitions, so the host hands each core
    pre-transposed views (layout permutation + the same fp16 RNE cast the
    SWDGE path would apply, done while sharding):
    xt = x^T [4096, 2048] fp16 and wt = W_shard^T [4096, 1376] int8.
  - Per-partition affine (scale, bias) is fused into the PSUM eviction.

Schedule (per core; measured ~323us vs the 303.8us pure PE-stream floor):
  - 1408 matmuls (4 s-chunks x 11 o-tiles x 32 k-slices), N=512 free dim,
    issue-limited at ~216ns each once the HAM clock gate is open.
  - Dependency-free warmup matmuls on raw (uninitialized) SBUF bridge from
    PE engine boot (~7us) to the first operand pair (~11.5us) so the HAM
    gate opens before real work and the PE never idles into a re-throttle.
  - x tiles stream as raw fp16 HWDGE loads on the sync queue; w k-slices
    stream as int8->fp16 SWDGE casts on the gpsimd queue (separate ~650ns
    per-descriptor pipelines). Both streams are self-chained with ramping
    depth (2 then 6): deep enough for full HBM bandwidth, ordered so tiles
    land in exactly consumption order (an unordered flood makes every
    startup transfer complete together at ~40us).
  - First w slice rides the scalar engine's HWDGE queue raw and is cast by
    the idle Vector engine, so the critical first pair's descriptors
    generate on two engines in parallel.
  - Chunk 0 uses 8+3 PSUM-bank groups (consumption 1.7us/k-slice exceeds
    the ~1.2us/k-slice ordered delivery, so the PE stays busy through the
    startup DMA window); later chunks 4+4+3; the last chunk 4+4+2+1 with
    the final eviction+store split into column halves to shorten the tail.
"""

import os
import numpy as np

import concourse.bass as bass
import concourse.tile as tile
from concourse import bacc, mybir
from concourse.bass_utils import run_bass_kernel_spmd

B = 1
S = 2048
I = 4096
O = 11008
N_CORES = 8
O_SHARD = O // N_CORES  # 1376
S_CHUNK = 512
P = 128


def build_bass(I_=I, O_SHARD_=O_SHARD, S_=S, S_CHUNK_=S_CHUNK):
    KT = I_ // P
    N_CHUNKS = S_ // S_CHUNK_
    OT = (O_SHARD_ + P - 1) // P

    MM_DT = mybir.dt.float16
    nc = bacc.Bacc("TRN2", target_bir_lowering=False, debug=False)

    # x arrives host-pre-cast to fp16 (same RNE rounding the SWDGE cast
    # would apply): halves the x HBM read volume and lets x tiles ride the
    # HWDGE queue as raw loads, keeping the SWDGE engine's serial
    # ~650ns/descriptor pipeline free for the weight casts.
    xt = nc.dram_tensor("xt", [I_, S_], mybir.dt.float16, kind="ExternalInput").ap()
    wt = nc.dram_tensor("wt", [I_, O_SHARD_], mybir.dt.int8, kind="ExternalInput").ap()
    # scale/bias come pre-arranged from the host as [P, OT] ([p, t] = value
    # for o = t*128 + p): a plain contiguous DMA instead of a gather
    # rearrange, whose 4 descriptor-generations (~700ns each, serial on the
    # SWDGE engine) would otherwise delay the critical first x/w transfers.
    scale = nc.dram_tensor("scale", [P, (O_SHARD_ + P - 1) // P], mybir.dt.float32, kind="ExternalInput").ap()
    bias = nc.dram_tensor("bias", [P, (O_SHARD_ + P - 1) // P], mybir.dt.float32, kind="ExternalInput").ap()
    yt = nc.dram_tensor("yt", [O_SHARD_, S_], mybir.dt.float32, kind="ExternalOutput").ap()

    with tile.TileContext(nc) as tc:
        with (
            tc.tile_pool(name="wres", bufs=1) as wres_pool,
            tc.tile_pool(name="consts", bufs=1) as const_pool,
            tc.tile_pool(name="xhilo", bufs=min(KT + 8, KT * N_CHUNKS)) as xhilo_pool,
            tc.tile_pool(name="outp", bufs=4) as out_pool,
            tc.tile_pool(name="psum", bufs=8, space="PSUM") as psum_pool,
        ):
            # Weight shard int8 -> fp16, kept resident in SBUF. One tile per
            # k-slice so matmuls only depend on their own slice. The
            # int8->fp16 cast happens inside the DMA (SWDGE path), so no
            # compute engine spends time on it.
            w_res = [None] * KT

            def emit_w(kt):
                w_kt = wres_pool.tile([P, O_SHARD_], MM_DT, tag=f"w{kt}")
                wd = nc.gpsimd.dma_start(w_kt[:], wt[kt * P:(kt + 1) * P, :])
                w_res[kt] = w_kt
                return wd

            # PE warm-up: dependency-free matmuls on a raw (uninitialized)
            # SBUF tensor keep the PE busy from engine boot, so the HAM
            # activity window opens the clock gate (K=8/8) before the first
            # real matmul and the PE never runs cold. Contents are
            # irrelevant: any bit pattern is valid fp16, and warm_ps is
            # never read. Raw allocation (not a pool tile) so the tile
            # framework imposes no write-before-read dependency.
            warm_sb = nc.alloc_sbuf_tensor("warm_sb_raw", [P, P], MM_DT).ap()
            warm_ps = psum_pool.tile([P, P], mybir.dt.float32, name="warm_ps", tag="psum")
            N_WARM = 42
            for i in range(N_WARM):
                nc.tensor.matmul(
                    warm_ps[:], warm_sb, warm_sb,
                    start=(i == 0), stop=(i == N_WARM - 1),
                )

            # per-partition scale/bias columns: [p, t] = value for o = t*128 + p
            # (host pre-arranged; single plain DMA each on the HWDGE queue;
            # emitted after chunk 0's conversions so their descriptors don't
            # delay the critical first x/w transfers)
            scale_t = const_pool.tile([P, OT], mybir.dt.float32)
            bias_t = const_pool.tile([P, OT], mybir.dt.float32)

            def emit_consts():
                nc.sync.dma_start(scale_t[:], scale[:, :])
                nc.sync.dma_start(bias_t[:], bias[:, :])

            # PSUM bank groups. Chunk 0 uses 8+3: during the startup DMA
            # window the PE burns ~1.7us per k-slice (8 o-tiles) while the
            # ordered DMA chains below deliver one (x, w) k-slice pair every
            # ~0.9-1.2us, so once the first pair lands the PE never starves.
            # Later chunks use 4+4+3 so two adjacent groups fit in the 8
            # banks and group transitions never wait on drains.
            def make_groups(sizes):
                groups, g0 = [], 0
                for gsz in sizes:
                    if g0 < OT:
                        groups.append((g0, min(g0 + gsz, OT)))
                        g0 += gsz
                return groups

            groups_chunk0 = make_groups((8, 3))
            groups_rest = make_groups((4, 4, 3))
            # Last chunk ends with a 1-tile group so the final
            # eviction+store chain after the last matmul is as short as
            # possible.
            groups_last = make_groups((4, 4, 2, 1))

            # Ordered DMA chains. The single SWDGE queue spreads descriptors
            # round-robin over 16 DMA engines, so an unordered flood makes
            # every startup transfer complete together at ~41us (PE idle,
            # HAM re-throttles). A tight depth-1 chain is as bad: one
            # descriptor in flight leaves 15 engines idle. So each stream
            # (x casts, w loads) is self-chained with RAMPING depth: depth 2
            # at the head (first tiles complete in ~1-2us), widening to 6
            # once the pipeline is primed (enough concurrency for full HBM
            # bandwidth, completion order skewed by at most 6 k-slices).
            all_casts = []
            all_w = []

            def chain(lst, dep_ins):
                n = len(lst)
                d = 4 if n < 6 else 6
                if n >= d:
                    bass._add_dep_helper(
                        dep_ins, lst[n - d].ins, sync=True,
                        reason="ordered DMA stream (ramping depth)",
                    )

            def emit_conversions(sc):
                s0 = sc * S_CHUNK_
                his = []
                for kt in range(KT):
                    xhi = xhilo_pool.tile([P, S_CHUNK_], MM_DT, tag="xhi")
                    # raw fp16 load on the sync engine's HWDGE queue: no
                    # compute engine and no SWDGE descriptor on the x path.
                    cd = nc.sync.dma_start(
                        xhi[:], xt[kt * P:(kt + 1) * P, s0:s0 + S_CHUNK_])
                    chain(all_casts, cd.ins)
                    all_casts.append(cd)
                    his.append(xhi)
                    if sc == 0:
                        if kt == 0:
                            # The first w slice rides the scalar engine's
                            # HWDGE queue (descriptor generates in parallel
                            # with x0's on sync and the w-stream's on
                            # gpsimd) as a raw int8 load, cast to fp16 by
                            # the otherwise-idle Vector engine.
                            w_raw = const_pool.tile([P, O_SHARD_], mybir.dt.int8, tag="w0raw")
                            wd = nc.scalar.dma_start(w_raw[:], wt[:P, :])
                            w_kt = wres_pool.tile([P, O_SHARD_], MM_DT, tag="w0")
                            nc.vector.tensor_copy(out=w_kt[:], in_=w_raw[:])
                            w_res[0] = w_kt
                        else:
                            wd = emit_w(kt)
                            chain(all_w, wd.ins)
                        all_w.append(wd)
                return (his,)

            def emit_groups(sc, his):
                # kt outer / o-tile inner: each x tile's last reader comes
                # early in the group sweep, so next-chunk conversions spread
                # over the whole chunk instead of bunching at its tail.
                s0 = sc * S_CHUNK_
                if sc == 0:
                    groups = groups_chunk0
                elif sc == N_CHUNKS - 1:
                    groups = groups_last
                else:
                    groups = groups_rest
                def emit_evict(ot, psum_ap, c0, c1):
                    orows = min(P, O_SHARD_ - ot * P)
                    out_t = out_pool.tile([P, c1 - c0], mybir.dt.float32)
                    nc.vector.tensor_scalar(
                        out=out_t[:orows, :],
                        in0=psum_ap,
                        scalar1=scale_t[:orows, ot:ot + 1],
                        scalar2=bias_t[:orows, ot:ot + 1],
                        op0=mybir.AluOpType.mult,
                        op1=mybir.AluOpType.add,
                    )
                    nc.sync.dma_start(
                        yt[ot * P:ot * P + orows, s0 + c0:s0 + c1],
                        out_t[:orows, :],
                    )

                for g_start, g_end in groups:
                    final_group = (sc == N_CHUNKS - 1) and (g_start, g_end) == groups[-1]
                    if final_group:
                        # The kernel tail is last-MM -> eviction -> store ->
                        # teardown. Split the final group's accumulation
                        # into column halves: half A's eviction and store
                        # overlap half B's matmuls, so only half a tile's
                        # eviction+store remains after the last matmul.
                        ot = g_start
                        orows = min(P, O_SHARD_ - ot * P)
                        half = S_CHUNK_ // 2
                        for c0, c1 in ((0, half), (half, S_CHUNK_)):
                            ps = psum_pool.tile(
                                [P, half], mybir.dt.float32,
                                name=f"psum_{sc}_{ot}_{c0}", tag="psum",
                            )
                            for kt in range(KT):
                                nc.tensor.matmul(
                                    ps[:orows, :], w_res[kt][:, ot * P:ot * P + orows],
                                    his[kt][:, c0:c1],
                                    start=(kt == 0), stop=(kt == KT - 1),
                                )
                            emit_evict(ot, ps[:orows, :], c0, c1)
                        continue
                    psums = {}
                    for ot in range(g_start, g_end):
                        psums[ot] = psum_pool.tile(
                            [P, S_CHUNK_], mybir.dt.float32,
                            name=f"psum_{sc}_{ot}", tag="psum",
                        )
                    for kt in range(KT):
                        for ot in range(g_start, g_end):
                            orows = min(P, O_SHARD_ - ot * P)
                            w_slice = w_res[kt][:, ot * P:ot * P + orows]
                            nc.tensor.matmul(
                                psums[ot][:orows, :], w_slice, his[kt][:],
                                start=(kt == 0), stop=(kt == KT - 1),
                            )
                    for ot in range(g_start, g_end):
                        emit_evict(ot, psums[ot][:min(P, O_SHARD_ - ot * P), :], 0, S_CHUNK_)

            # Software-pipelined emission: conversions for chunk sc+1 are
            # emitted before chunk sc's matmul groups, so in the per-engine
            # FIFO streams next-chunk subs/casts sit ahead of this chunk's
            # PSUM drains.
            prev = emit_conversions(0)
            emit_consts()
            for sc in range(N_CHUNKS):
                if sc + 1 < N_CHUNKS:
                    nxt = emit_conversions(sc + 1)
                else:
                    nxt = None
                emit_groups(sc, *prev)
                prev = nxt

    nc.compile()
    return nc


_NC_CACHE = None


def _get_nc():
    global _NC_CACHE
    if _NC_CACHE is None:
        _NC_CACHE = build_bass()
    return _NC_CACHE


def run(inputs, trace=False, trace_cores=None, tmpdir=None):
    x = np.asarray(inputs["x"])
    w = np.asarray(inputs["weight_int8"])
    scale = np.asarray(inputs["scale"], dtype=np.float32)
    bias = np.asarray(inputs["bias"], dtype=np.float32)

    if w.dtype != np.int8:
        w = w.astype(np.int8)
    x2d = np.ascontiguousarray(x.reshape(S, I).astype(np.float32, copy=False))
    xtr = np.ascontiguousarray(x2d.T.astype(np.float16))  # [I, S] fp16

    OT = (O_SHARD + P - 1) // P

    def prearrange(v):
        # [O_SHARD] -> [P, OT] with [p, t] = v[t*128 + p] (zero padded)
        out = np.zeros(OT * P, dtype=np.float32)
        out[:O_SHARD] = v
        return np.ascontiguousarray(out.reshape(OT, P).T)

    in_maps = []
    for c in range(N_CORES):
        sl = slice(c * O_SHARD, (c + 1) * O_SHARD)
        in_maps.append({
            "xt": xtr,
            "wt": np.ascontiguousarray(w[sl, :].T),  # [I, O_SHARD]
            "scale": prearrange(scale[sl]),
            "bias": prearrange(bias[sl]),
        })

    nc = _get_nc()
    kwargs = {}
    if trace:
        kwargs["trace"] = True
        if trace_cores is not None:
            kwargs["trace_cores"] = trace_cores
        if tmpdir is not None:
            kwargs["tmpdir"] = tmpdir
    res = run_bass_kernel_spmd(nc, in_maps, core_ids=list(range(N_CORES)), **kwargs)

    yt_full = np.concatenate([res.results[c]["yt"] for c in range(N_CORES)], axis=0)
    out = np.ascontiguousarray(yt_full.T).reshape(B, S, O).astype(np.float32, copy=False)
    if trace:
        return out, res
    return out


def kernel(**inputs) -> np.ndarray:
    return run(inputs, trace=False)



# revision 57
# speedup vs baseline: 1.0021x; 1.0021x over previous
"""CompressedLinear (int8 weight, per-row scale) on 8 Trainium2 NeuronCores.

Math: y[b,s,o] = sum_i x[b,s,i] * (w_int8[o,i] * scale[o]) + bias[o]

Strategy (tensor-parallel over out_features, per sharding hint):
  - Shard W/scale/bias rows across 8 cores (1376 rows each); x replicated.
  - Scale is applied to the matmul OUTPUT (algebraically identical), so the
    device matmuls run on the raw int8 weights cast to fp16 (int8 is exact
    in fp16).
  - Single fp16 matmul pass: int8 weights are exact in fp16, and casting x
    to fp16 bounds the output relative error at ~2e-4 (Frobenius) while
    halving the PE work vs a hi/lo two-pass scheme. (fp8 DoubleRow cannot
    beat this: representing int8 W exactly needs 2 fp8 passes, and e4m3 x
    alone costs ~3.6% error — over the 2e-2 gate — so a 3rd pass would be
    needed; 2+ passes at the measured 1.44x lose to 1 pass at 1x.)
  - Each core computes yT[o_shard, s] = W_shard @ x^T; both operands need
    the contraction dim on SBUF partitions, so the host hands each core
    pre-transposed views (layout permutation + the same fp16 RNE cast the
    SWDGE path would apply, done while sharding):
    xt = x^T [4096, 2048] fp16 and wt = W_shard^T [4096, 1376] int8.
  - Per-partition affine (scale, bias) is fused into the PSUM eviction.

Schedule (per core; measured ~321.4us vs the ~304us pure PE-stream floor):
  - 1408 matmuls (4 s-chunks x 11 o-tiles x 32 k-slices), N=512 free dim,
    issue-limited at ~216ns each once the HAM clock gate is open.
  - Dependency-free warmup matmuls on raw (uninitialized) SBUF bridge from
    PE engine boot (~7us) to the first operand pair (~11.5us) so the HAM
    gate opens before real work and the PE never idles into a re-throttle.
  - x tiles stream as raw fp16 HWDGE loads on the sync queue; w k-slices
    stream as int8->fp16 SWDGE casts on the gpsimd queue (separate ~650ns
    per-descriptor pipelines). Both streams are self-chained with ramping
    depth (4 then 6): deep enough for full HBM bandwidth, ordered so tiles
    land in approximately consumption order (an unordered flood makes every
    startup transfer complete together at ~40us; a depth-2 head measured
    ~1.3us slower from chunk-0 delivery jitter).
  - First w slice rides the scalar engine's HWDGE queue raw and is cast by
    the idle Vector engine, so the critical first pair's descriptors
    generate on two engines in parallel.
  - Chunk 0 uses 8+3 PSUM-bank groups (consumption 1.7us/k-slice exceeds
    the ~1.2us/k-slice ordered delivery, so the PE stays busy through the
    startup DMA window); later chunks 4+4+3; the last chunk 4+4+2+1 with
    the final eviction+store split into column halves to shorten the tail.
"""

import os
import numpy as np

import concourse.bass as bass
import concourse.tile as tile
from concourse import bacc, mybir
from concourse.bass_utils import run_bass_kernel_spmd

B = 1
S = 2048
I = 4096
O = 11008
N_CORES = 8
O_SHARD = O // N_CORES  # 1376
S_CHUNK = 512
P = 128


def build_bass(I_=I, O_SHARD_=O_SHARD, S_=S, S_CHUNK_=S_CHUNK):
    KT = I_ // P
    N_CHUNKS = S_ // S_CHUNK_
    OT = (O_SHARD_ + P - 1) // P

    MM_DT = mybir.dt.float16
    nc = bacc.Bacc("TRN2", target_bir_lowering=False, debug=False)

    # x arrives host-pre-cast to fp16 (same RNE rounding the SWDGE cast
    # would apply): halves the x HBM read volume and lets x tiles ride the
    # HWDGE queue as raw loads, keeping the SWDGE engine's serial
    # ~650ns/descriptor pipeline free for the weight casts.
    xt = nc.dram_tensor("xt", [I_, S_], mybir.dt.float16, kind="ExternalInput").ap()
    wt = nc.dram_tensor("wt", [I_, O_SHARD_], mybir.dt.int8, kind="ExternalInput").ap()
    # scale/bias come pre-arranged from the host as [P, OT] ([p, t] = value
    # for o = t*128 + p): a plain contiguous DMA instead of a gather
    # rearrange, whose 4 descriptor-generations (~700ns each, serial on the
    # SWDGE engine) would otherwise delay the critical first x/w transfers.
    scale = nc.dram_tensor("scale", [P, (O_SHARD_ + P - 1) // P], mybir.dt.float32, kind="ExternalInput").ap()
    bias = nc.dram_tensor("bias", [P, (O_SHARD_ + P - 1) // P], mybir.dt.float32, kind="ExternalInput").ap()
    yt = nc.dram_tensor("yt", [O_SHARD_, S_], mybir.dt.float32, kind="ExternalOutput").ap()

    with tile.TileContext(nc) as tc:
        with (
            tc.tile_pool(name="wres", bufs=1) as wres_pool,
            tc.tile_pool(name="consts", bufs=1) as const_pool,
            tc.tile_pool(name="xhilo", bufs=min(KT + 8, KT * N_CHUNKS)) as xhilo_pool,
            tc.tile_pool(name="outp", bufs=4) as out_pool,
            tc.tile_pool(name="psum", bufs=8, space="PSUM") as psum_pool,
        ):
            # Weight shard int8 -> fp16, kept resident in SBUF. One tile per
            # k-slice so matmuls only depend on their own slice. The
            # int8->fp16 cast happens inside the DMA (SWDGE path), so no
            # compute engine spends time on it.
            w_res = [None] * KT

            def emit_w(kt):
                w_kt = wres_pool.tile([P, O_SHARD_], MM_DT, tag=f"w{kt}")
                wd = nc.gpsimd.dma_start(w_kt[:], wt[kt * P:(kt + 1) * P, :])
                w_res[kt] = w_kt
                return wd

            # PE warm-up: dependency-free matmuls on a raw (uninitialized)
            # SBUF tensor keep the PE busy from engine boot, so the HAM
            # activity window opens the clock gate (K=8/8) before the first
            # real matmul and the PE never runs cold. Contents are
            # irrelevant: any bit pattern is valid fp16, and warm_ps is
            # never read. Raw allocation (not a pool tile) so the tile
            # framework imposes no write-before-read dependency.
            warm_sb = nc.alloc_sbuf_tensor("warm_sb_raw", [P, P], MM_DT).ap()
            warm_ps = psum_pool.tile([P, P], mybir.dt.float32, name="warm_ps", tag="psum")
            N_WARM = 42
            for i in range(N_WARM):
                nc.tensor.matmul(
                    warm_ps[:], warm_sb, warm_sb,
                    start=(i == 0), stop=(i == N_WARM - 1),
                )

            # per-partition scale/bias columns: [p, t] = value for o = t*128 + p
            # (host pre-arranged; single plain DMA each on the HWDGE queue;
            # emitted after chunk 0's conversions so their descriptors don't
            # delay the critical first x/w transfers)
            scale_t = const_pool.tile([P, OT], mybir.dt.float32)
            bias_t = const_pool.tile([P, OT], mybir.dt.float32)

            def emit_consts():
                nc.sync.dma_start(scale_t[:], scale[:, :])
                nc.sync.dma_start(bias_t[:], bias[:, :])

            # PSUM bank groups. Chunk 0 uses 8+3: during the startup DMA
            # window the PE burns ~1.7us per k-slice (8 o-tiles) while the
            # ordered DMA chains below deliver one (x, w) k-slice pair every
            # ~0.9-1.2us, so once the first pair lands the PE never starves.
            # Later chunks use 4+4+3 so two adjacent groups fit in the 8
            # banks and group transitions never wait on drains.
            def make_groups(sizes):
                groups, g0 = [], 0
                for gsz in sizes:
                    if g0 < OT:
                        groups.append((g0, min(g0 + gsz, OT)))
                        g0 += gsz
                return groups

            groups_chunk0 = make_groups((8, 3))
            groups_rest = make_groups((4, 4, 3))
            # Last chunk ends with a 1-tile group so the final
            # eviction+store chain after the last matmul is as short as
            # possible.
            groups_last = make_groups((4, 4, 2, 1))

            # Ordered DMA chains. The single SWDGE queue spreads descriptors
            # round-robin over 16 DMA engines, so an unordered flood makes
            # every startup transfer complete together at ~41us (PE idle,
            # HAM re-throttles). A tight depth-1 chain is as bad: one
            # descriptor in flight leaves 15 engines idle. So each stream
            # (x casts, w loads) is self-chained with RAMPING depth: depth 2
            # at the head (first tiles complete in ~1-2us), widening to 6
            # once the pipeline is primed (enough concurrency for full HBM
            # bandwidth, completion order skewed by at most 6 k-slices).
            all_casts = []
            all_w = []

            def chain(lst, dep_ins):
                n = len(lst)
                d = 4 if n < 6 else 6
                if n >= d:
                    bass._add_dep_helper(
                        dep_ins, lst[n - d].ins, sync=True,
                        reason="ordered DMA stream (ramping depth)",
                    )

            def emit_conversions(sc):
                s0 = sc * S_CHUNK_
                his = []
                for kt in range(KT):
                    xhi = xhilo_pool.tile([P, S_CHUNK_], MM_DT, tag="xhi")
                    # raw fp16 load on the sync engine's HWDGE queue: no
                    # compute engine and no SWDGE descriptor on the x path.
                    cd = nc.sync.dma_start(
                        xhi[:], xt[kt * P:(kt + 1) * P, s0:s0 + S_CHUNK_])
                    chain(all_casts, cd.ins)
                    all_casts.append(cd)
                    his.append(xhi)
                    if sc == 0:
                        if kt == 0:
                            # The first w slice rides the scalar engine's
                            # HWDGE queue (descriptor generates in parallel
                            # with x0's on sync and the w-stream's on
                            # gpsimd) as a raw int8 load, cast to fp16 by
                            # the otherwise-idle Vector engine.
                            w_raw = const_pool.tile([P, O_SHARD_], mybir.dt.int8, tag="w0raw")
                            wd = nc.scalar.dma_start(w_raw[:], wt[:P, :])
                            w_kt = wres_pool.tile([P, O_SHARD_], MM_DT, tag="w0")
                            nc.vector.tensor_copy(out=w_kt[:], in_=w_raw[:])
                            w_res[0] = w_kt
                        else:
                            wd = emit_w(kt)
                            chain(all_w, wd.ins)
                        all_w.append(wd)
                return (his,)

            def emit_groups(sc, his):
                # kt outer / o-tile inner: each x tile's last reader comes
                # early in the group sweep, so next-chunk conversions spread
                # over the whole chunk instead of bunching at its tail.
                s0 = sc * S_CHUNK_
                if sc == 0:
                    groups = groups_chunk0
                elif sc == N_CHUNKS - 1:
                    groups = groups_last
                else:
                    groups = groups_rest
                def emit_evict(ot, psum_ap, c0, c1):
                    orows = min(P, O_SHARD_ - ot * P)
                    out_t = out_pool.tile([P, c1 - c0], mybir.dt.float32)
                    nc.vector.tensor_scalar(
                        out=out_t[:orows, :],
                        in0=psum_ap,
                        scalar1=scale_t[:orows, ot:ot + 1],
                        scalar2=bias_t[:orows, ot:ot + 1],
                        op0=mybir.AluOpType.mult,
                        op1=mybir.AluOpType.add,
                    )
                    nc.sync.dma_start(
                        yt[ot * P:ot * P + orows, s0 + c0:s0 + c1],
                        out_t[:orows, :],
                    )

                for g_start, g_end in groups:
                    final_group = (sc == N_CHUNKS - 1) and (g_start, g_end) == groups[-1]
                    if final_group:
                        # The kernel tail is last-MM -> eviction -> store ->
                        # teardown. Split the final group's accumulation
                        # into column halves: half A's eviction and store
                        # overlap half B's matmuls, so only half a tile's
                        # eviction+store remains after the last matmul.
                        ot = g_start
                        orows = min(P, O_SHARD_ - ot * P)
                        half = S_CHUNK_ // 2
                        for c0, c1 in ((0, half), (half, S_CHUNK_)):
                            ps = psum_pool.tile(
                                [P, half], mybir.dt.float32,
                                name=f"psum_{sc}_{ot}_{c0}", tag="psum",
                            )
                            for kt in range(KT):
                                nc.tensor.matmul(
                                    ps[:orows, :], w_res[kt][:, ot * P:ot * P + orows],
                                    his[kt][:, c0:c1],
                                    start=(kt == 0), stop=(kt == KT - 1),
                                )
                            emit_evict(ot, ps[:orows, :], c0, c1)
                        continue
                    psums = {}
                    for ot in range(g_start, g_end):
                        psums[ot] = psum_pool.tile(
                            [P, S_CHUNK_], mybir.dt.float32,
                            name=f"psum_{sc}_{ot}", tag="psum",
                        )
                    for kt in range(KT):
                        for ot in range(g_start, g_end):
                            orows = min(P, O_SHARD_ - ot * P)
                            w_slice = w_res[kt][:, ot * P:ot * P + orows]
                            nc.tensor.matmul(
                                psums[ot][:orows, :], w_slice, his[kt][:],
                                start=(kt == 0), stop=(kt == KT - 1),
                            )
                    for ot in range(g_start, g_end):
                        emit_evict(ot, psums[ot][:min(P, O_SHARD_ - ot * P), :], 0, S_CHUNK_)

            # Software-pipelined emission: conversions for chunk sc+1 are
            # emitted before chunk sc's matmul groups, so in the per-engine
            # FIFO streams next-chunk subs/casts sit ahead of this chunk's
            # PSUM drains.
            prev = emit_conversions(0)
            emit_consts()
            for sc in range(N_CHUNKS):
                if sc + 1 < N_CHUNKS:
                    nxt = emit_conversions(sc + 1)
                else:
                    nxt = None
                emit_groups(sc, *prev)
                prev = nxt

    nc.compile()
    return nc


_NC_CACHE = None


def _get_nc():
    global _NC_CACHE
    if _NC_CACHE is None:
        _NC_CACHE = build_bass()
    return _NC_CACHE


def run(inputs, trace=False, trace_cores=None, tmpdir=None):
    x = np.asarray(inputs["x"])
    w = np.asarray(inputs["weight_int8"])
    scale = np.asarray(inputs["scale"], dtype=np.float32)
    bias = np.asarray(inputs["bias"], dtype=np.float32)

    if w.dtype != np.int8:
        w = w.astype(np.int8)
    x2d = np.ascontiguousarray(x.reshape(S, I).astype(np.float32, copy=False))
    xtr = np.ascontiguousarray(x2d.T.astype(np.float16))  # [I, S] fp16

    OT = (O_SHARD + P - 1) // P

    def prearrange(v):
        # [O_SHARD] -> [P, OT] with [p, t] = v[t*128 + p] (zero padded)
        out = np.zeros(OT * P, dtype=np.float32)
        out[:O_SHARD] = v
        return np.ascontiguousarray(out.reshape(OT, P).T)

    in_maps = []
    for c in range(N_CORES):
        sl = slice(c * O_SHARD, (c + 1) * O_SHARD)
        in_maps.append({
            "xt": xtr,
            "wt": np.ascontiguousarray(w[sl, :].T),  # [I, O_SHARD]
            "scale": prearrange(scale[sl]),
            "bias": prearrange(bias[sl]),
        })

    nc = _get_nc()
    kwargs = {}
    if trace:
        kwargs["trace"] = True
        if trace_cores is not None:
            kwargs["trace_cores"] = trace_cores
        if tmpdir is not None:
            kwargs["tmpdir"] = tmpdir
    res = run_bass_kernel_spmd(nc, in_maps, core_ids=list(range(N_CORES)), **kwargs)

    yt_full = np.concatenate([res.results[c]["yt"] for c in range(N_CORES)], axis=0)
    out = np.ascontiguousarray(yt_full.T).reshape(B, S, O).astype(np.float32, copy=False)
    if trace:
        return out, res
    return out


def kernel(**inputs) -> np.ndarray:
    return run(inputs, trace=False)

